# revision 6
# baseline (speedup 1.0000x reference)
"""Trainium2 Bass kernel for ConstituencyMFVI.

Reference computation (per batch b):
    mask2o[i,j,k] = mask[i,j] & (min(i,j) != k) & (max(i,j) != k)
    A = s_pair * mask2o                       # [L, L, L] per (b, i): A_i = [j, k]
    q = s_span
    repeat 3x:  q[i,j] = s_span[i,j] + sum_k A[i,j,k] * sigmoid(q)[i,k]
    out = sigmoid(q)

Device strategy (pure data-parallel over batch, 4 batches/core on 8 cores):
  - Host marshals inputs: applies mask2o, transposes s_pair to [b, k, i, j]
    (so each pair's A_i^T = lhsT[k, j] is a contiguous SBUF slice), casts to
    fp16 (11-bit mantissa; |A|<6, sigmoid in [0,1] -> ~1e-3 final abs err),
    and transposes s_span to [b, j, i].
  - Device per (b, iter): 128 matvecs on TensorE — stationary = A_i^T
    ([128k, 128j] fp16, FWL), moving = sigmoid column [128k, 1], each writing
    column i of a [128j, 128i] fp32 PSUM tile (one accumulation group,
    disjoint columns). Then one DVE add (+ s_span^T) and one ScalarE sigmoid
    per sweep. Batches are round-robined so PE never waits on the
    sigmoid turnaround.
"""

import numpy as np

import concourse.bacc as bacc
import concourse.bass as bass
import concourse.mybir as mybir
import concourse.tile as tile
from concourse.bass_utils import run_bass_kernel_spmd

N_CORES = 8
B, L = 32, 128
BPC = B // N_CORES  # batches per core
MAX_ITER = 3

_cached = {}


def build_nc(repeats=1):
    nc = bacc.Bacc("TRN2", target_bir_lowering=False, debug=False)
    sp = nc.dram_tensor("sp", [BPC, L, L, L], mybir.dt.float16, kind="ExternalInput")
    ss = nc.dram_tensor("ss", [BPC, L, L], mybir.dt.float32, kind="ExternalInput")
    out = nc.dram_tensor("out", [BPC, L, L], mybir.dt.float32, kind="ExternalOutput")

    with tile.TileContext(nc) as tc:
        with (
            tc.tile_pool(name="atp", bufs=1) as atp,
            tc.tile_pool(name="misc", bufs=2) as misc,
            tc.tile_pool(name="spool", bufs=2) as spool,
            tc.tile_pool(name="qpool", bufs=2, space="PSUM") as qpool,
        ):
            for r in range(repeats):
                at = []
                sst = []
                s_cur = [None] * BPC
                for b in range(BPC):
                    a = atp.tile(
                        [L, L, L], mybir.dt.float16, name=f"at{b}_{r}", tag=f"at{b}",
                        bufs=1,
                    )
                    nc.sync.dma_start(a[:], sp[b])
                    at.append(a)
                    s0 = misc.tile(
                        [L, L], mybir.dt.float32, name=f"sst{b}_{r}", tag=f"sst{b}"
                    )
                    nc.sync.dma_start(s0[:], ss[b])
                    sst.append(s0)
                # S0 = sigmoid(s_span^T)
                for b in range(BPC):
                    sc = spool.tile(
                        [L, L], mybir.dt.float16, name=f"s{b}_i_{r}", tag=f"s{b}"
                    )
                    nc.scalar.activation(
                        sc[:], sst[b][:], mybir.ActivationFunctionType.Sigmoid
                    )
                    s_cur[b] = sc
                for t in range(MAX_ITER):
                    last = t == MAX_ITER - 1
                    for b in range(BPC):
                        q = qpool.tile(
                            [L, L], mybir.dt.float32, name=f"q{b}_{t}_{r}", tag=f"q{b}"
                        )
                        for i in range(L):
                            nc.tensor.matmul(
                                q[:, i : i + 1],
                                at[b][:, i, :],
                                s_cur[b][:, i : i + 1],
                                start=(i == 0),
                                stop=(i == L - 1),
                            )
                        qs = misc.tile(
                            [L, L], mybir.dt.float32, name=f"qs{b}_{t}_{r}",
                            tag=f"qs{b}",
                        )
                        nc.vector.tensor_add(qs[:], q[:], sst[b][:])
                        sn = spool.tile(
                            [L, L],
                            mybir.dt.float32 if last else mybir.dt.float16,
                            name=f"s{b}_{t}_{r}",
                            tag=f"sf{b}" if last else f"s{b}",
                        )
                        nc.scalar.activation(
                            sn[:], qs[:], mybir.ActivationFunctionType.Sigmoid
                        )
                        if last:
                            nc.sync.dma_start(out[b], sn[:])
                        s_cur[b] = sn
    nc.compile()
    return nc


def _prep(s_span, s_pair, mask):
    """Host-side marshalling: mask2o fold, transpose, cast."""
    s_span = np.asarray(s_span, dtype=np.float32)
    s_pair = np.asarray(s_pair, dtype=np.float32)
    mask = np.asarray(mask)
    sp = s_pair * mask[:, :, :, None].astype(np.float32)
    idx = np.arange(L)
    ii, jj = np.meshgrid(idx, idx, indexing="ij")
    ls = np.minimum(ii, jj)
    rs = np.maximum(ii, jj)
    sp[:, ii, jj, ls] = 0.0
    sp[:, ii, jj, rs] = 0.0
    # [b, i, j, k] -> [b, k, i, j], fp16
    spT = np.ascontiguousarray(sp.transpose(0, 3, 1, 2)).astype(np.float16)
    # [b, i, j] -> [b, j, i]
    ssT = np.ascontiguousarray(s_span.transpose(0, 2, 1))
    return spT, ssT


def kernel(s_span, s_pair, mask):
    if "nc" not in _cached:
        _cached["nc"] = build_nc()
    nc = _cached["nc"]

    spT, ssT = _prep(s_span, s_pair, mask)

    in_maps = []
    for c in range(N_CORES):
        lo, hi = c * BPC, (c + 1) * BPC
        in_maps.append(
            {
                "sp": np.ascontiguousarray(spT[lo:hi]),
                "ss": np.ascontiguousarray(ssT[lo:hi]),
            }
        )

    res = run_bass_kernel_spmd(nc, in_maps, core_ids=list(range(N_CORES)))
    outs = [r["out"] for r in res.results]  # each [BPC, L(j), L(i)]
    full = np.concatenate(outs, axis=0)  # [B, j, i]
    return np.ascontiguousarray(full.transpose(0, 2, 1)).astype(np.float32)


# revision 7
# speedup vs baseline: 39.9704x; 39.9704x over previous
"""Trainium2 Bass kernel for ConstituencyMFVI.

Reference computation (per batch b):
    mask2o[i,j,k] = mask[i,j] & (min(i,j) != k) & (max(i,j) != k)
    A = s_pair * mask2o                       # [L, L, L]; per (b,i): A_i = [j, k]
    q = s_span
    repeat 3x:  q[i,j] = s_span[i,j] + sum_k A[i,j,k] * sigmoid(q)[i,k]
    out = sigmoid(q)

Strategy: pure data parallel over batch (4 batches per core on 8 cores).
Host marshals inputs (folds the deterministic mask2o, casts to fp16 —
11-bit mantissa keeps final abs err ~4e-3). The device computes each MFVI
iteration per batch with two fat DVE instructions in natural layout:

    T[i, j, k] = SP[i, j, k] * V[i, k]     (tensor_mul, V broadcast over j
                                            via a stride-0 AP dim)
    qred[i, j] = sum_k T[i, j, k]          (tensor_reduce axis=X)
    q = qred + s_span; V' = sigmoid(q)     (DVE add + ScalarE sigmoid)

This execution environment is dominated by fixed per-instruction overhead
(~80us/instruction measured, regardless of instruction size), so the kernel
minimizes instruction count (~56 total) with maximally large operations.
"""

import numpy as np

import concourse.bacc as bacc
import concourse.mybir as mybir
import concourse.tile as tile
from concourse.bass_utils import run_bass_kernel_spmd

N_CORES = 8
B, L = 32, 128
BPC = B // N_CORES  # batches per core
MAX_ITER = 3

_cached = {}


def build_nc(repeats=1):
    nc = bacc.Bacc("TRN2", target_bir_lowering=False, debug=False)
    sp = nc.dram_tensor("sp", [BPC, L, L, L], mybir.dt.float16, kind="ExternalInput")
    ss = nc.dram_tensor("ss", [BPC, L, L], mybir.dt.float32, kind="ExternalInput")
    out = nc.dram_tensor("out", [BPC, L, L], mybir.dt.float32, kind="ExternalOutput")

    with tile.TileContext(nc) as tc:
        with (
            tc.tile_pool(name="atp", bufs=1) as atp,
            tc.tile_pool(name="tp", bufs=1) as tp,
            tc.tile_pool(name="misc", bufs=2) as misc,
        ):
            for r in range(repeats):
                # SP: [i(part), b, j, k] fp16 — one DMA for all batches.
                spt = atp.tile(
                    [L, BPC, L, L], mybir.dt.float16, name=f"spt_{r}", tag="spt",
                )
                nc.sync.dma_start(spt[:], sp[:].transpose([1, 0, 2, 3]))
                # s_span: [i(part), b, j] f32 — one DMA.
                sst = misc.tile(
                    [L, BPC, L], mybir.dt.float32, name=f"sst_{r}", tag="sst"
                )
                nc.sync.dma_start(sst[:], ss[:].transpose([1, 0, 2]))

                # V0 = sigmoid(s_span), all batches in one ACT op: [i, b, k] fp16
                v = misc.tile([L, BPC, L], mybir.dt.float16, name=f"v0_{r}", tag="v")
                nc.scalar.activation(
                    v[:], sst[:], mybir.ActivationFunctionType.Sigmoid
                )

                for t in range(MAX_ITER):
                    last = t == MAX_ITER - 1
                    qred = misc.tile(
                        [L, BPC, L], mybir.dt.float32, name=f"qred_{t}_{r}", tag="qred"
                    )
                    for b in range(BPC):
                        tmp = tp.tile(
                            [L, L, L], mybir.dt.float16, name=f"tmp{b}_{t}_{r}",
                            tag="tmp",
                        )
                        nc.vector.tensor_mul(
                            tmp[:],
                            spt[:, b],
                            v[:, b].unsqueeze(1).broadcast_to([L, L, L]),
                        )
                        nc.vector.tensor_reduce(
                            qred[:, b],
                            tmp[:],
                            axis=mybir.AxisListType.X,
                            op=mybir.AluOpType.add,
                        )
                    q = misc.tile(
                        [L, BPC, L], mybir.dt.float32, name=f"q_{t}_{r}", tag="q"
                    )
                    nc.vector.tensor_add(q[:], qred[:], sst[:])
                    v = misc.tile(
                        [L, BPC, L],
                        mybir.dt.float32 if last else mybir.dt.float16,
                        name=f"v_{t}_{r}",
                        tag="vf" if last else "v",
                    )
                    nc.scalar.activation(
                        v[:], q[:], mybir.ActivationFunctionType.Sigmoid
                    )
                # out dram [b, i, j] <- v [i(part), b, j]
                nc.sync.dma_start(out[:].transpose([1, 0, 2]), v[:])
    nc.compile()
    return nc


def _prep(s_span, s_pair, mask):
    """Host-side marshalling: mask2o fold + fp16 cast (natural layout)."""
    s_span = np.ascontiguousarray(np.asarray(s_span, dtype=np.float32))
    s_pair = np.asarray(s_pair, dtype=np.float32)
    mask = np.asarray(mask)
    sp = s_pair * mask[:, :, :, None].astype(np.float32)
    idx = np.arange(L)
    ii, jj = np.meshgrid(idx, idx, indexing="ij")
    sp[:, ii, jj, np.minimum(ii, jj)] = 0.0
    sp[:, ii, jj, np.maximum(ii, jj)] = 0.0
    return sp.astype(np.float16), s_span


def kernel(s_span, s_pair, mask):
    if "nc" not in _cached:
        _cached["nc"] = build_nc()
    nc = _cached["nc"]

    sp16, ss32 = _prep(s_span, s_pair, mask)

    in_maps = []
    for c in range(N_CORES):
        lo, hi = c * BPC, (c + 1) * BPC
        in_maps.append(
            {
                "sp": np.ascontiguousarray(sp16[lo:hi]),
                "ss": np.ascontiguousarray(ss32[lo:hi]),
            }
        )

    res = run_bass_kernel_spmd(nc, in_maps, core_ids=list(range(N_CORES)))
    outs = [r["out"] for r in res.results]  # each [BPC, L, L]
    return np.ascontiguousarray(np.concatenate(outs, axis=0)).astype(np.float32)
